# revision 2
# baseline (speedup 1.0000x reference)
"""KKT loss kernel v5 for Trainium2 (Bass/Tile), 8 NeuronCores.

Host converts each sparse system to jagged ELL (classes K in {16,32,64,128},
G = 128/K bins per column; a bin's K slots sit on K consecutive partitions of
one column) and expands the gather operands x[cols] / lam[rows] into the same
layout. Device pipeline per problem-side:

  DVE:  product tile pt = v * g            [128, SLOTF] bf16, one op
  PE :  block-ones matmuls reduce each 128-column stripe into
        PSUM[32u : 32u+G, 128w : 128w+128]  (9 stripes per PSUM bank)
  ACT:  evacuate PSUM -> SBUF f32 (one copy per PSUM tile)
  DMA:  compact quadrant rows -> row-sum tile axs [RT, 128] f32
  epilogue runs directly in that stacked layout; b / lam / c are packed by
  the host into the identical [RT, 128] layout; dummy capacity rows are
  zero so they contribute nothing. Per-problem partials fold via a final
  ones-matmul.
"""

import os
import sys

import numpy as np
import ml_dtypes

sys.path.insert(0, "/opt/trn_rl_repo")

# ---------------------------------------------------------------------------
# Environment shims (self-contained): this container's walrus build encodes at
# most one semaphore wait/update per instruction, and the image's antenv lacks
# the axon NTFF profile hook. Both are patched here at import time.
# ---------------------------------------------------------------------------
import json as _json
import types as _types


def _split_sync(bir):
    for fn in bir.get("functions", []):
        for blk in fn.get("blocks", []):
            out = []
            for ins in blk.get("instructions", []):
                si = ins.get("sync_info")
                if not si:
                    out.append(ins)
                    continue
                waits = si.get("on_wait") or []
                ups = si.get("on_update") or []
                if len(waits) > 1:
                    for j, w in enumerate(waits[:-1]):
                        out.append({
                            "debug": ins.get("debug", 0),
                            "engine": ins["engine"],
                            "ins": [],
                            "name": f"{ins['name']}_w{j}",
                            "opcode": "EventSemaphore",
                            "outs": [],
                            "sync_info": {"on_update": [], "on_wait": [w]},
                        })
                    si["on_wait"] = waits[-1:]
                out.append(ins)
                if len(ups) > 1:
                    si["on_update"] = ups[:1]
                    for j, u in enumerate(ups[1:]):
                        out.append({
                            "debug": ins.get("debug", 0),
                            "engine": ins["engine"],
                            "ins": [],
                            "name": f"{ins['name']}_u{j}",
                            "opcode": "EventSemaphore",
                            "outs": [],
                            "sync_info": {"on_update": [u], "on_wait": []},
                        })
            blk["instructions"] = out
    return bir


def _install_shims():
    from concourse import bass_utils, bass2jax

    if not getattr(bass_utils, "_bir_fix_installed", False):
        orig = bass_utils.compile_bir_kernel

        def patched(bir_json, tmpdir, neff_name="file.neff"):
            fixed = _json.dumps(_split_sync(_json.loads(bir_json))).encode()
            return orig(fixed, tmpdir, neff_name=neff_name)

        bass_utils.compile_bir_kernel = patched
        bass2jax.compile_bir_kernel = patched
        bass_utils._bir_fix_installed = True

    import antenv

    try:
        from antenv import axon_hooks  # noqa: F401
    except ImportError:
        mod = _types.ModuleType("antenv.axon_hooks")
        mod._hook = None
        mod.set_axon_ntff_profile_hook = lambda h: setattr(mod, "_hook", h)
        mod.get_axon_ntff_profile_hook = lambda: mod._hook
        sys.modules["antenv.axon_hooks"] = mod
        antenv.axon_hooks = mod
        try:
            from trn_agent_boot.trn_boot import _ntff_profile_via_ctypes

            hook = _ntff_profile_via_ctypes("/opt/axon/libaxon_pjrt.so")
            if hook is not None:
                mod.set_axon_ntff_profile_hook(hook)
        except Exception:
            pass


_install_shims()

from contextlib import ExitStack

import concourse.bass as bass
import concourse.mybir as mybir
from concourse import tile
from concourse.bass_utils import run_bass_kernel_spmd

B, M, N, NNZ = 64, 8192, 8192, 262144
W_PRIMAL, W_DUAL, W_STAT, W_COMP = 0.1, 0.1, 0.6, 0.2

PB = 8
NCORES = 8
CLASSES = (16, 32, 64, 128)

f32 = mybir.dt.float32
bf16 = mybir.dt.bfloat16

LAST_EXEC_NS = None
LAST_RES = None
_CACHED = {}


def _geom(rlocs):
    """segs: (K, G, rl, F, S, soff, roff); stripes of all classes flattened."""
    segs = []
    so = ro = 0
    for K, rl in zip(CLASSES, rlocs):
        if rl <= 0:
            continue
        G = 128 // K
        rl = G * ((rl + G - 1) // G)
        F = rl * K
        S = rl // G
        segs.append((K, G, rl, F, S, so, ro))
        so += F
        ro += rl
    return segs, so, ro


def _stripes(segs):
    """Flat stripe list: (ci, s_in_class, G, soff, roff)."""
    out = []
    for ci, (K, G, rl, F, S, soff, roff) in enumerate(segs):
        for s in range(S):
            out.append((ci, s, G, K, soff, roff, rl))
    return out


def build_kernel(rlocs):
    nc = bass.Bass()
    segs, SLOTF, RT = _geom(rlocs)
    NCl = len(segs)
    stripes = _stripes(segs)
    NT = (len(stripes) + 8) // 9  # psum tiles per side

    vr = nc.dram_tensor("vr", [PB, 128, SLOTF], bf16, kind="ExternalInput")
    xr = nc.dram_tensor("xr", [PB, 128, SLOTF], bf16, kind="ExternalInput")
    vc = nc.dram_tensor("vc", [PB, 128, SLOTF], bf16, kind="ExternalInput")
    lc = nc.dram_tensor("lc", [PB, 128, SLOTF], bf16, kind="ExternalInput")
    bp = nc.dram_tensor("bp", [PB, RT, 128], f32, kind="ExternalInput")
    cp = nc.dram_tensor("cp", [PB, RT, 128], f32, kind="ExternalInput")
    lr = nc.dram_tensor("lr", [PB, RT, 128], f32, kind="ExternalInput")
    NS = len(stripes)
    stones = nc.dram_tensor("stones", [128, NS * 128], bf16, kind="ExternalInput")
    out = nc.dram_tensor("out", [1, 4 * PB], f32, kind="ExternalOutput")

    with tile.TileContext(nc) as tc, ExitStack() as ctx:
        const = ctx.enter_context(tc.tile_pool(name="const", bufs=1))
        sa = ctx.enter_context(tc.tile_pool(name="sa", bufs=2))
        sb = ctx.enter_context(tc.tile_pool(name="sb", bufs=2))
        work = ctx.enter_context(tc.tile_pool(name="work", bufs=2))
        pspool = ctx.enter_context(tc.tile_pool(name="pspool", bufs=2, space="PSUM"))
        psfin = ctx.enter_context(tc.tile_pool(name="psfin", bufs=1, space="PSUM"))

        stats = const.tile([128, 4 * PB], f32, tag="stats")
        nc.vector.memset(stats[:], 0.0)
        ones = const.tile([128, 1], f32, tag="ones")
        nc.vector.memset(ones[:], 1.0)
        st_ones = const.tile([128, NS * 128], bf16, tag="st_ones")
        nc.sync.dma_start(st_ones[:], stones[:])

        def side(j, vten, gten, tagp, pool):
            vt = pool.tile([128, SLOTF], bf16, tag=f"v{tagp}")
            nc.sync.dma_start(vt[:], vten[j])
            gt = pool.tile([128, SLOTF], bf16, tag=f"g{tagp}")
            nc.sync.dma_start(gt[:], gten[j])
            pt = pool.tile([128, SLOTF], bf16, tag=f"p{tagp}")
            nc.vector.tensor_tensor(pt[:], vt[:], gt[:], mybir.AluOpType.mult)

            ps = pspool.tile(
                [128, 128], f32, tag=f"ps{tagp}", name=f"ps{tagp}", bufs=2
            )
            for k, (ci, sic, G, K, soff, roff, rlc) in enumerate(stripes):
                nc.tensor.matmul(
                    ps[:],
                    st_ones[:, 128 * k : 128 * (k + 1)],
                    pt[:, soff + 128 * sic : soff + 128 * (sic + 1)],
                    start=(k == 0),
                    stop=(k == len(stripes) - 1),
                    skip_group_check=True,
                )
            return ps

        for j in range(PB):
            axs = side(j, vr, xr, "a", sa)
            btile = work.tile([RT, 128], f32, tag="btile")
            nc.sync.dma_start(btile[:], bp[j])
            ltile = work.tile([RT, 128], f32, tag="ltile")
            nc.sync.dma_start(ltile[:], lr[j])

            d = work.tile([RT, 128], f32, tag="d")
            nc.vector.tensor_tensor(d[:], axs[0:RT, :], btile[:], mybir.AluOpType.subtract)
            rd = work.tile([RT, 128], f32, tag="rd")
            nc.vector.tensor_scalar(rd[:], d[:], 0.0, None, mybir.AluOpType.max)
            sq = work.tile([RT, 128], f32, tag="sq")
            nc.scalar.activation(
                sq[:], rd[:], mybir.ActivationFunctionType.Square,
                accum_out=stats[0:RT, 4 * j : 4 * j + 1],
            )
            ld = work.tile([RT, 128], f32, tag="ld")
            nc.vector.tensor_tensor(ld[:], ltile[:], d[:], mybir.AluOpType.mult)
            sq2 = work.tile([RT, 128], f32, tag="sq2")
            nc.scalar.activation(
                sq2[:], ld[:], mybir.ActivationFunctionType.Square,
                accum_out=stats[0:RT, 4 * j + 1 : 4 * j + 2],
            )

            ats = side(j, vc, lc, "b", sb)
            ctile = work.tile([RT, 128], f32, tag="ctile")
            nc.sync.dma_start(ctile[:], cp[j])
            st = work.tile([RT, 128], f32, tag="st")
            nc.vector.tensor_tensor(st[:], ats[0:RT, :], ctile[:], mybir.AluOpType.add)
            sq3 = work.tile([RT, 128], f32, tag="sq3")
            nc.scalar.activation(
                sq3[:], st[:], mybir.ActivationFunctionType.Square,
                accum_out=stats[0:RT, 4 * j + 2 : 4 * j + 3],
            )
            mn = work.tile([RT, 128], f32, tag="mn")
            nc.vector.tensor_scalar(mn[:], ltile[:], 0.0, None, mybir.AluOpType.min)
            sq4 = work.tile([RT, 128], f32, tag="sq4")
            nc.scalar.activation(
                sq4[:], mn[:], mybir.ActivationFunctionType.Square,
                accum_out=stats[0:RT, 4 * j + 3 : 4 * j + 4],
            )

        ps = psfin.tile([1, 4 * PB], f32, tag="psf")
        nc.tensor.matmul(ps[:], ones[:], stats[:], start=True, stop=True)
        res = const.tile([1, 4 * PB], f32, tag="res")
        nc.scalar.copy(res[:], ps[:])
        nc.sync.dma_start(out[:], res[:])

    return nc


def _side_caps(idx_arr):
    mx = np.zeros(len(CLASSES), dtype=np.int64)
    for i in range(idx_arr.shape[0]):
        counts = np.bincount(idx_arr[i], minlength=M)
        if counts.max() > CLASSES[-1]:
            raise ValueError("bin count exceeds largest class")
        cid = np.searchsorted(CLASSES, counts, side="left")
        sizes = np.bincount(cid, minlength=len(CLASSES))
        mx = np.maximum(mx, sizes)
    return mx


def _pack_side(idx, oidx, vals, gvec, vecs, rlocs):
    segs, SLOTF, RT = _geom(rlocs)
    counts = np.bincount(idx, minlength=M)
    cid = np.searchsorted(CLASSES, counts, side="left")

    order_bins = np.argsort(cid, kind="stable")
    csizes = np.bincount(cid, minlength=len(CLASSES))
    cstart = np.concatenate(([0], np.cumsum(csizes)[:-1]))
    rank = np.empty(M, dtype=np.int64)
    rank[order_bins] = np.arange(M) - cstart[cid[order_bins]]

    Karr = np.zeros(len(CLASSES), np.int64)
    Garr = np.ones(len(CLASSES), np.int64)
    soffa = np.zeros(len(CLASSES), np.int64)
    roffa = np.zeros(len(CLASSES), np.int64)
    for K, G, rl, F, S, so, ro in segs:
        c = CLASSES.index(K)
        Karr[c], Garr[c], soffa[c], roffa[c] = K, G, so, ro

    order = np.argsort(idx, kind="stable")
    sidx = idx[order]
    starts = np.concatenate(([0], np.cumsum(counts)[:-1]))
    slot = np.arange(NNZ) - starts[sidx]
    r = rank[sidx]
    g = r % Garr[cid[sidx]]
    dest_p = g * Karr[cid[sidx]] + slot
    dest_c = soffa[cid[sidx]] + r // Garr[cid[sidx]]

    v_stream = np.zeros((128, SLOTF), dtype=ml_dtypes.bfloat16)
    g_stream = np.zeros((128, SLOTF), dtype=ml_dtypes.bfloat16)
    v_stream[dest_p, dest_c] = vals[order].astype(ml_dtypes.bfloat16)
    g_stream[dest_p, dest_c] = gvec[oidx[order]].astype(ml_dtypes.bfloat16)

    # vectors in the stacked row-sum layout:
    # bin rank r: g=r%G, f=r//G, stripe s=f//128, col c=f%128
    # -> axs row = roff + s*G + g, col = c
    binmap = np.full((RT, 128), -1, dtype=np.int64)
    Gb = Garr[cid]
    gb = rank % Gb
    fb = rank // Gb
    pm = roffa[cid] + (fb // 128) * Gb + gb
    cm = fb % 128
    binmap[pm, cm] = np.arange(M)
    outv = []
    for v in vecs:
        pv = np.where(binmap >= 0, v[np.clip(binmap, 0, M - 1)], 0.0).astype(
            np.float32
        )
        outv.append(pv)
    return v_stream, g_stream, outv


def make_stones(rlocs):
    segs, _, _ = _geom(rlocs)
    stripes = _stripes(segs)
    stones = np.zeros((128, len(stripes) * 128), dtype=ml_dtypes.bfloat16)
    for k, (ci, sic, G, K, soff, roff, rl) in enumerate(stripes):
        for g in range(G):
            stones[g * K : (g + 1) * K, 128 * k + roff + sic * G + g] = 1.0
    return stones


def prepare_inputs(x, lam, A_vals, A_rows, A_cols, b_pad, c_pad):
    rl_r = _side_caps(A_rows)
    rl_c = _side_caps(A_cols)
    rlocs = tuple(int(np.ceil(max(a, b) / 128)) for a, b in zip(rl_r, rl_c))
    segs, SLOTF, RT = _geom(rlocs)
    stones = make_stones(rlocs)

    in_maps = []
    for core in range(NCORES):
        mp = {
            "vr": np.empty((PB, 128, SLOTF), dtype=ml_dtypes.bfloat16),
            "xr": np.empty((PB, 128, SLOTF), dtype=ml_dtypes.bfloat16),
            "vc": np.empty((PB, 128, SLOTF), dtype=ml_dtypes.bfloat16),
            "lc": np.empty((PB, 128, SLOTF), dtype=ml_dtypes.bfloat16),
            "bp": np.empty((PB, RT, 128), dtype=np.float32),
            "cp": np.empty((PB, RT, 128), dtype=np.float32),
            "lr": np.empty((PB, RT, 128), dtype=np.float32),
            "stones": stones,
        }
        for j in range(PB):
            i = PB * core + j
            v, g, vecs = _pack_side(
                A_rows[i], A_cols[i], A_vals[i], x[i], [b_pad[i], lam[i]], rlocs
            )
            mp["vr"][j], mp["xr"][j] = v, g
            mp["bp"][j], mp["lr"][j] = vecs
            v, g, vecs = _pack_side(
                A_cols[i], A_rows[i], A_vals[i], lam[i], [c_pad[i]], rlocs
            )
            mp["vc"][j], mp["lc"][j] = v, g
            mp["cp"][j] = vecs[0]
        in_maps.append(mp)
    return in_maps, rlocs


def kernel(x_hat, lam_hat, A_vals, A_rows, A_cols, b_pad, c_pad):
    global LAST_EXEC_NS, LAST_RES
    x = np.asarray(x_hat, dtype=np.float32).reshape(B, N)
    lam = np.asarray(lam_hat, dtype=np.float32).reshape(B, M)
    A_vals = np.asarray(A_vals, dtype=np.float32)
    A_rows = np.asarray(A_rows, dtype=np.int32)
    A_cols = np.asarray(A_cols, dtype=np.int32)
    b_pad = np.asarray(b_pad, dtype=np.float32)
    c_pad = np.asarray(c_pad, dtype=np.float32)

    try:
        in_maps, rlocs = prepare_inputs(
            x, lam, A_vals, A_rows, A_cols, b_pad, c_pad
        )
        if ("nc", rlocs) not in _CACHED:
            _CACHED[("nc", rlocs)] = build_kernel(rlocs)
        res = run_bass_kernel_spmd(
            _CACHED[("nc", rlocs)],
            in_maps,
            core_ids=list(range(NCORES)),
            trace=bool(int(os.environ.get("KKT_TRACE", "0"))),
        )
        LAST_EXEC_NS = res.exec_time_ns
        LAST_RES = res
    except Exception:
        import traceback

        traceback.print_exc()
        return _host_fallback(x, lam, A_vals, A_rows, A_cols, b_pad, c_pad)

    total = np.float64(0.0)
    for i in range(NCORES):
        v = np.asarray(res.results[i]["out"], dtype=np.float64).reshape(4 * PB)
        for j in range(PB):
            prim, comp, stat, dual = v[4 * j : 4 * j + 4]
            total += (
                W_PRIMAL * prim / M
                + W_COMP * comp / M
                + W_STAT * stat / N
                + W_DUAL * dual / M
            )
    return np.float32(total / B)


def _host_fallback(x, lam, vals, rows, cols, b_pad, c_pad):
    tot = 0.0
    for i in range(B):
        Ax = np.bincount(
            rows[i], weights=(vals[i] * x[i][cols[i]]).astype(np.float64), minlength=M
        )
        ATl = np.bincount(
            cols[i], weights=(vals[i] * lam[i][rows[i]]).astype(np.float64), minlength=N
        )
        d = Ax - b_pad[i]
        tot += (
            W_PRIMAL * np.mean(np.maximum(d, 0.0) ** 2)
            + W_DUAL * np.mean(np.maximum(-lam[i], 0.0) ** 2)
            + W_STAT * np.mean((ATl + c_pad[i]) ** 2)
            + W_COMP * np.mean((lam[i] * d) ** 2)
        )
    return np.float32(tot / B)
